# revision 2
# baseline (speedup 1.0000x reference)
"""GCN (3-layer GraphConv, norm='right') Trainium2 Bass kernel.

Strategy: single NeuronCore, single launch. Per layer:
  gather y[src] rows (256B each) from a DRAM table via dma_gather,
  aggregate per 128-dst-node block with one-hot S-matrix matmuls into PSUM
  (inv_deg folded into S), epilogue applies bias/relu and the next layer's
  projection, writing the next gather table.

Edges are grouped by dst block and split into two streams by src half
(A: src<25088, B: src>=25088) because dma_gather indices are int16.
Per-(block,stream) edge lists are padded to multiples of 128; padding
edges carry slot=999 (matches no dst slot) and inv_deg=0, so they
contribute exactly zero.
"""
import numpy as np

import concourse.bass as bass
import concourse.tile as tile
from concourse import bacc, mybir
from concourse.bass_utils import run_bass_kernel_spmd

N_NODES = 50000
N_EDGES = 800000
IN_FEATS, F, N_CLASSES = 128, 64, 40
NBLK = (N_NODES + 127) // 128          # 391
NROWS = NBLK * 128                     # 50048
HSPLIT = 25088                         # rows [0,HSPLIT) -> stream A
TPC = 64                               # tiles per gather chunk (8192 idxs)
ACT_EVERY = 4                          # every 4th tile's S built on ScalarE

_cache = {}


def _pack_stream(srcv, slotv, invdv, blkv, nblk, base):
    """Pad per-block edge groups to multiples of 128 tiles; return arrays."""
    cnt = np.bincount(blkv, minlength=nblk)
    tiles = (cnt + 127) // 128
    T = int(tiles.sum())
    starts = np.concatenate([[0], np.cumsum(cnt)[:-1]])
    tile_starts = np.concatenate([[0], np.cumsum(tiles)[:-1]])
    idx_pad = np.zeros(T * 128, dtype=np.int16)
    slot_pad = np.full(T * 128, 999.0, dtype=np.float32)
    invd_pad = np.zeros(T * 128, dtype=np.float32)
    if len(srcv):
        rank = np.arange(len(srcv)) - np.repeat(starts, cnt)
        pos = np.repeat(tile_starts * 128, cnt) + rank
        idx_pad[pos] = (srcv - base).astype(np.int16)
        slot_pad[pos] = slotv
        invd_pad[pos] = invdv
    # idx dram layout: index i of the stream at [i%16, i//16], replicated x8
    idx_dram = np.tile(idx_pad.reshape(-1, 16).T, (8, 1)).copy()  # [128, T*8]
    slot_t = slot_pad.reshape(T, 128).T.copy()                    # [128, T]
    invd_t = invd_pad.reshape(T, 128).T.copy()
    return idx_dram, slot_t, invd_t, tiles, tile_starts, T


def _prep(features, src, dst, W0, b0, W1, b1, W2, b2):
    deg = np.bincount(dst, minlength=N_NODES).astype(np.float32)
    invd = (1.0 / np.maximum(deg, 1.0)).astype(np.float32)

    order = np.argsort(dst, kind="stable")
    dst_s = dst[order].astype(np.int64)
    src_s = src[order].astype(np.int64)
    blk = dst_s // 128
    slot = (dst_s % 128).astype(np.float32)
    invd_e = invd[dst_s]

    am = src_s < HSPLIT
    A = _pack_stream(src_s[am], slot[am], invd_e[am], blk[am], NBLK, 0)
    B = _pack_stream(src_s[~am], slot[~am], invd_e[~am], blk[~am], NBLK, HSPLIT)

    xT = np.zeros((IN_FEATS, NROWS), dtype=np.float32)
    xT[:, :N_NODES] = np.ascontiguousarray(features.T)

    W2p = np.zeros((F, F), dtype=np.float32)
    W2p[:, :N_CLASSES] = W2[:, :N_CLASSES]
    b2p = np.zeros((F, 1), dtype=np.float32)
    b2v = np.asarray(b2).reshape(-1)
    b2p[:min(len(b2v), F), 0] = b2v[:min(len(b2v), F)]

    in_map = {
        "xT": xT,
        "W0": np.ascontiguousarray(W0.astype(np.float32)),
        "W1": np.ascontiguousarray(W1.astype(np.float32)),
        "W2p": W2p,
        "b0": np.asarray(b0, dtype=np.float32).reshape(F, 1),
        "b1": np.asarray(b1, dtype=np.float32).reshape(F, 1),
        "b2p": b2p,
        "iota": np.tile(np.arange(128, dtype=np.float32), (128, 1)),
        "ident": np.eye(128, dtype=np.float32),
        "idxA": A[0], "slotA": A[1], "invdA": A[2], "ninvdA": -A[2],
        "idxB": B[0], "slotB": B[1], "invdB": B[2], "ninvdB": -B[2],
    }
    sched = {"A": (A[3], A[4], A[5]), "B": (B[3], B[4], B[5])}
    return in_map, sched


def _build(sched):
    TA = sched["A"][2]
    TB = sched["B"][2]

    nc = bacc.Bacc("TRN2", num_devices=1, dynamic_dma_scratch_size=65536)
    dt = mybir.dt.float32

    xT_in = nc.dram_tensor("xT", [IN_FEATS, NROWS], dt, kind="ExternalInput")
    W0_in = nc.dram_tensor("W0", [IN_FEATS, F], dt, kind="ExternalInput")
    W1_in = nc.dram_tensor("W1", [F, F], dt, kind="ExternalInput")
    W2_in = nc.dram_tensor("W2p", [F, F], dt, kind="ExternalInput")
    b0_in = nc.dram_tensor("b0", [F, 1], dt, kind="ExternalInput")
    b1_in = nc.dram_tensor("b1", [F, 1], dt, kind="ExternalInput")
    b2_in = nc.dram_tensor("b2p", [F, 1], dt, kind="ExternalInput")
    iota_in = nc.dram_tensor("iota", [128, 128], dt, kind="ExternalInput")
    ident_in = nc.dram_tensor("ident", [128, 128], dt, kind="ExternalInput")
    meta_in = {}
    for s, T in (("A", TA), ("B", TB)):
        meta_in["idx" + s] = nc.dram_tensor("idx" + s, [128, max(T, 1) * 8], mybir.dt.int16, kind="ExternalInput")
        for nm in ("slot", "invd", "ninvd"):
            meta_in[nm + s] = nc.dram_tensor(nm + s, [128, max(T, 1)], dt, kind="ExternalInput")
    out = nc.dram_tensor("out", [NROWS, F], dt, kind="ExternalOutput")

    with tile.TileContext(nc) as tc:
        with tc.tile_pool(name="const", bufs=1) as cp, \
             tc.tile_pool(name="dram", bufs=1, space="DRAM") as dram, \
             tc.tile_pool(name="msg", bufs=2) as mp, \
             tc.tile_pool(name="midx", bufs=2) as ip, \
             tc.tile_pool(name="marr", bufs=2) as ap_, \
             tc.tile_pool(name="stl", bufs=4) as sp, \
             tc.tile_pool(name="xblk", bufs=2) as xp, \
             tc.tile_pool(name="ep", bufs=3) as epp, \
             tc.tile_pool(name="agg", bufs=3, space="PSUM") as pp, \
             tc.tile_pool(name="eps", bufs=4, space="PSUM") as pp2:

            iota_t = cp.tile([128, 128], dt)
            nc.sync.dma_start(iota_t[:], iota_in[:])
            ident_t = cp.tile([128, 128], dt)
            nc.sync.dma_start(ident_t[:], ident_in[:])
            W0_t = cp.tile([IN_FEATS, F], dt)
            nc.sync.dma_start(W0_t[:], W0_in[:])
            W1_t = cp.tile([F, F], dt)
            nc.sync.dma_start(W1_t[:], W1_in[:])
            W2_t = cp.tile([F, F], dt)
            nc.sync.dma_start(W2_t[:], W2_in[:])
            b0_t = cp.tile([F, 1], dt)
            nc.sync.dma_start(b0_t[:], b0_in[:])
            b1_t = cp.tile([F, 1], dt)
            nc.sync.dma_start(b1_t[:], b1_in[:])
            b2_t = cp.tile([F, 1], dt)
            nc.sync.dma_start(b2_t[:], b2_in[:])

            tbl = []
            for l in range(3):
                tb = dram.tile([NROWS, F], dt, tag=f"t{l}")
                tbl.append(tb)

            # ---- Layer-1 projection: t0 = X @ W0 ----
            for b in range(NBLK):
                xb = xp.tile([IN_FEATS, 128], dt, tag="xb")
                nc.sync.dma_start(xb[:], xT_in[:, b * 128:(b + 1) * 128])
                yp = pp2.tile([128, F], dt, tag="eps")
                nc.tensor.matmul(yp[:], xb[:], W0_t[:], start=True, stop=True)
                ys = epp.tile([128, F], dt, tag="ysb")
                nc.scalar.copy(ys[:], yp[:])
                nc.sync.dma_start(tbl[0][b * 128:(b + 1) * 128, :], ys[:])

            # ---- Layers ----
            tile_ctr = 0
            for l in range(3):
                table = tbl[l]
                views = {"A": table[0:HSPLIT, :], "B": table[HSPLIT:NROWS, :]}
                msgs = {}
                arrs = {}
                for s in ("A", "B"):
                    T = sched[s][2]
                    n_chunks = (T + TPC - 1) // TPC
                    msgs[s] = []
                    arrs[s] = []
                    for ch in range(n_chunks):
                        nt = min(TPC, T - ch * TPC)
                        idx_t = ip.tile([128, nt * 8], mybir.dt.int16, tag="idx" + s)
                        nc.sync.dma_start(idx_t[:], meta_in["idx" + s][:, ch * TPC * 8: ch * TPC * 8 + nt * 8])
                        sl = ap_.tile([128, nt], dt, tag="sl" + s)
                        nc.sync.dma_start(sl[:], meta_in["slot" + s][:, ch * TPC: ch * TPC + nt])
                        iv = ap_.tile([128, nt], dt, tag="iv" + s)
                        nc.sync.dma_start(iv[:], meta_in["invd" + s][:, ch * TPC: ch * TPC + nt])
                        nv = ap_.tile([128, nt], dt, tag="nv" + s)
                        nc.sync.dma_start(nv[:], meta_in["ninvd" + s][:, ch * TPC: ch * TPC + nt])
                        msg = mp.tile([128, nt, F], dt, tag="msg" + s)
                        nc.gpsimd.dma_gather(
                            msg[:], views[s], idx_t[:],
                            num_idxs=nt * 128, num_idxs_reg=nt * 128,
                            elem_size=F, single_packet=False)
                        msgs[s].append(msg)
                        arrs[s].append((sl, iv, nv))

                for b in range(NBLK):
                    refs = []
                    for s in ("A", "B"):
                        tiles, tstarts, _T = sched[s]
                        for t in range(int(tstarts[b]), int(tstarts[b] + tiles[b])):
                            refs.append((s, t // TPC, t % TPC))
                    agg = pp.tile([128, F], dt, tag="agg")
                    nt_b = len(refs)
                    for i, (s, ch, col) in enumerate(refs):
                        sl, iv, nv = arrs[s][ch]
                        S = sp.tile([128, 128], dt, tag="S")
                        tile_ctr += 1
                        if tile_ctr % ACT_EVERY == 0:
                            S1 = sp.tile([128, 128], dt, tag="S1")
                            nc.scalar.activation(
                                S1[:], iota_t[:], mybir.ActivationFunctionType.Abs,
                                bias=sl[:, col:col + 1], scale=-1.0)
                            nc.scalar.activation(
                                S[:], S1[:], mybir.ActivationFunctionType.Relu,
                                bias=iv[:, col:col + 1], scale=nv[:, col:col + 1])
                        else:
                            nc.vector.tensor_scalar(
                                S[:], iota_t[:], sl[:, col:col + 1], iv[:, col:col + 1],
                                mybir.AluOpType.is_equal, mybir.AluOpType.mult)
                        nc.tensor.matmul(agg[:], S[:], msgs[s][ch][:, col, :],
                                         start=(i == 0), stop=(i == nt_b - 1))

                    # epilogue
                    t0 = epp.tile([128, F], dt, tag="t0")
                    if nt_b == 0:
                        nc.vector.memset(t0[:], 0.0)
                    else:
                        nc.scalar.copy(t0[:], agg[:])
                    t0T = pp2.tile([F, 128], dt, tag="eps")
                    nc.tensor.transpose(t0T[:], t0[:], ident_t[:])
                    rows = slice(b * 128, (b + 1) * 128)
                    if l == 0:
                        hT = epp.tile([F, 128], dt, tag="hT")
                        nc.scalar.activation(hT[:], t0T[:], mybir.ActivationFunctionType.Relu,
                                             bias=b0_t[:, 0:1], scale=1.0)
                        yT = pp2.tile([F, 128], dt, tag="eps")
                        nc.tensor.matmul(yT[:], W1_t[:], hT[:], start=True, stop=True)
                        yTs = epp.tile([F, 128], dt, tag="yTs")
                        nc.scalar.copy(yTs[:], yT[:])
                        yps = pp2.tile([128, F], dt, tag="eps")
                        nc.tensor.transpose(yps[:], yTs[:], ident_t[0:F, 0:F])
                        ysb = epp.tile([128, F], dt, tag="ysb")
                        nc.scalar.copy(ysb[:], yps[:])
                        nc.sync.dma_start(tbl[1][rows, :], ysb[:])
                    elif l == 1:
                        hT = epp.tile([F, 128], dt, tag="hT")
                        nc.scalar.activation(hT[:], t0T[:], mybir.ActivationFunctionType.Relu,
                                             bias=b1_t[:, 0:1], scale=1.0)
                        hps = pp2.tile([128, F], dt, tag="eps")
                        nc.tensor.transpose(hps[:], hT[:], ident_t[0:F, 0:F])
                        hsb = epp.tile([128, F], dt, tag="ysb")
                        nc.scalar.copy(hsb[:], hps[:])
                        nc.sync.dma_start(tbl[2][rows, :], hsb[:])
                    else:
                        # out = aggT.T @ W2p + b2: project the (normalized) agg
                        aT = epp.tile([F, 128], dt, tag="hT")
                        nc.scalar.copy(aT[:], t0T[:])
                        oT = pp2.tile([F, 128], dt, tag="eps")
                        nc.tensor.matmul(oT[:], W2_t[:], aT[:], start=True, stop=True)
                        oTb = epp.tile([F, 128], dt, tag="yTs")
                        nc.scalar.activation(oTb[:], oT[:], mybir.ActivationFunctionType.Identity,
                                             bias=b2_t[:, 0:1], scale=1.0)
                        ops_ = pp2.tile([128, F], dt, tag="eps")
                        nc.tensor.transpose(ops_[:], oTb[:], ident_t[0:F, 0:F])
                        osb = epp.tile([128, F], dt, tag="ysb")
                        nc.scalar.copy(osb[:], ops_[:])
                        nc.sync.dma_start(out[rows, :], osb[:])

    nc.compile()
    return nc


def kernel(features, src, dst, W0, b0, W1, b1, W2, b2):
    features = np.asarray(features, dtype=np.float32)
    src = np.asarray(src).astype(np.int64)
    dst = np.asarray(dst).astype(np.int64)
    in_map, sched = _prep(features, src, dst,
                          np.asarray(W0), np.asarray(b0), np.asarray(W1),
                          np.asarray(b1), np.asarray(W2), np.asarray(b2))
    key = (sched["A"][2], sched["B"][2],
           tuple(sched["A"][0].tolist()), tuple(sched["B"][0].tolist()))
    if _cache.get("key") != key:
        _cache["nc"] = _build(sched)
        _cache["key"] = key
    nc = _cache["nc"]
    res = run_bass_kernel_spmd(nc, [in_map], core_ids=[0])
    full = res.results[0]["out"]
    return np.ascontiguousarray(full[:N_NODES, :N_CLASSES])


# revision 8
# speedup vs baseline: 1.3192x; 1.3192x over previous
"""GCN (3-layer GraphConv, norm='right') Trainium2 Bass kernel.

Strategy: single NeuronCore, single launch. Per layer:
  gather y[src] rows (256B each) from a DRAM table via dma_gather,
  aggregate per 128-dst-node block with one-hot S-matrix matmuls into PSUM
  (inv_deg folded into S), epilogue applies bias/relu and the next layer's
  projection, writing the next gather table.

Edges are grouped by dst block and split into two streams by src half
(A: src<25088, B: src>=25088) because dma_gather indices are int16.
Per-(block,stream) edge lists are padded to multiples of 128; padding
edges carry slot=999 (matches no dst slot) and inv_deg=0, so they
contribute exactly zero.
"""
import os as _os
import numpy as np

import concourse.bass as bass
import concourse.tile as tile
from concourse import bacc, mybir
from concourse.bass_utils import run_bass_kernel_spmd

N_NODES = 50000
N_EDGES = 800000
IN_FEATS, F, N_CLASSES = 128, 64, 40
NBLK = (N_NODES + 127) // 128          # 391
NROWS = NBLK * 128                     # 50048
HSPLIT = 25088                         # rows [0,HSPLIT) -> stream A
TPC = 16                               # tiles per gather chunk (4096 idxs)
ACT_EVERY = 10 ** 9                    # S-builds stay on VectorE (ACT is slower)

_cache = {}


def _pack_stream(srcv, slotv, invdv, blkv, nblk, base):
    """Pad per-block edge groups to multiples of 128 tiles; return arrays."""
    cnt = np.bincount(blkv, minlength=nblk)
    tiles = (cnt + 127) // 128
    T = int(tiles.sum())
    starts = np.concatenate([[0], np.cumsum(cnt)[:-1]])
    tile_starts = np.concatenate([[0], np.cumsum(tiles)[:-1]])
    idx_pad = np.zeros(T * 128, dtype=np.int16)
    slot_pad = np.full(T * 128, 999.0, dtype=np.float32)
    invd_pad = np.zeros(T * 128, dtype=np.float32)
    if len(srcv):
        rank = np.arange(len(srcv)) - np.repeat(starts, cnt)
        pos = np.repeat(tile_starts * 128, cnt) + rank
        idx_pad[pos] = (srcv - base).astype(np.int16)
        slot_pad[pos] = slotv
        invd_pad[pos] = invdv
    # idx dram layout: index i of the stream at [i%16, i//16], replicated x8
    idx_dram = np.tile(idx_pad.reshape(-1, 16).T, (8, 1)).copy()  # [128, T*8]
    slot_t = slot_pad.reshape(T, 128).T.copy()                    # [128, T]
    invd_t = invd_pad.reshape(T, 128).T.copy()
    return idx_dram, slot_t, invd_t, tiles, tile_starts, T


def _meta3(S):
    """Per-chunk-interleaved [slot | invd | ninvd] array: [128, 3*T]."""
    T = S[5]
    out = np.empty((128, 3 * max(T, 1)), dtype=np.float32)
    for ch in range((T + TPC - 1) // TPC):
        nt = min(TPC, T - ch * TPC)
        base = 3 * ch * TPC
        out[:, base:base + nt] = S[1][:, ch * TPC:ch * TPC + nt]
        out[:, base + nt:base + 2 * nt] = S[2][:, ch * TPC:ch * TPC + nt]
        out[:, base + 2 * nt:base + 3 * nt] = -S[2][:, ch * TPC:ch * TPC + nt]
    return np.ascontiguousarray(out)


def _prep(features, src, dst, W0, b0, W1, b1, W2, b2):
    deg = np.bincount(dst, minlength=N_NODES).astype(np.float32)
    invd = (1.0 / np.maximum(deg, 1.0)).astype(np.float32)

    order = np.argsort(dst, kind="stable")
    dst_s = dst[order].astype(np.int64)
    src_s = src[order].astype(np.int64)
    blk = dst_s // 128
    slot = (dst_s % 128).astype(np.float32)
    invd_e = invd[dst_s]

    am = src_s < HSPLIT
    A = _pack_stream(src_s[am], slot[am], invd_e[am], blk[am], NBLK, 0)
    B = _pack_stream(src_s[~am], slot[~am], invd_e[~am], blk[~am], NBLK, HSPLIT)

    xT = np.zeros((IN_FEATS, NROWS), dtype=np.float32)
    xT[:, :N_NODES] = np.ascontiguousarray(features.T)

    W2p = np.zeros((F, F), dtype=np.float32)
    W2p[:, :N_CLASSES] = W2[:, :N_CLASSES]
    b2p = np.zeros((F, 1), dtype=np.float32)
    b2v = np.asarray(b2).reshape(-1)
    b2p[:min(len(b2v), F), 0] = b2v[:min(len(b2v), F)]

    in_map = {
        "xT": xT,
        "W0": np.ascontiguousarray(W0.astype(np.float32)),
        "W1": np.ascontiguousarray(W1.astype(np.float32)),
        "W2p": W2p,
        "b0": np.asarray(b0, dtype=np.float32).reshape(F, 1),
        "b1": np.asarray(b1, dtype=np.float32).reshape(F, 1),
        "b2p": b2p,
        "iota": np.tile(np.arange(128, dtype=np.float32), (128, 1)),
        "ident": np.eye(128, dtype=np.float32),
        "idxA": A[0], "metaA": _meta3(A),
        "idxB": B[0], "metaB": _meta3(B),
    }
    sched = {"A": (A[3], A[4], A[5]), "B": (B[3], B[4], B[5])}
    return in_map, sched


def _build(sched):
    TA = sched["A"][2]
    TB = sched["B"][2]

    nc = bacc.Bacc("TRN2", num_devices=1, dynamic_dma_scratch_size=65536)
    dt = mybir.dt.float32

    xT_in = nc.dram_tensor("xT", [IN_FEATS, NROWS], dt, kind="ExternalInput")
    W0_in = nc.dram_tensor("W0", [IN_FEATS, F], dt, kind="ExternalInput")
    W1_in = nc.dram_tensor("W1", [F, F], dt, kind="ExternalInput")
    W2_in = nc.dram_tensor("W2p", [F, F], dt, kind="ExternalInput")
    b0_in = nc.dram_tensor("b0", [F, 1], dt, kind="ExternalInput")
    b1_in = nc.dram_tensor("b1", [F, 1], dt, kind="ExternalInput")
    b2_in = nc.dram_tensor("b2p", [F, 1], dt, kind="ExternalInput")
    iota_in = nc.dram_tensor("iota", [128, 128], dt, kind="ExternalInput")
    ident_in = nc.dram_tensor("ident", [128, 128], dt, kind="ExternalInput")
    meta_in = {}
    for s, T in (("A", TA), ("B", TB)):
        meta_in["idx" + s] = nc.dram_tensor("idx" + s, [128, max(T, 1) * 8], mybir.dt.int16, kind="ExternalInput")
        meta_in["meta" + s] = nc.dram_tensor("meta" + s, [128, max(T, 1) * 3], dt, kind="ExternalInput")
    out = nc.dram_tensor("out", [NROWS, F], dt, kind="ExternalOutput")

    with tile.TileContext(nc) as tc:
        with tc.tile_pool(name="const", bufs=1) as cp, \
             tc.tile_pool(name="dram", bufs=1, space="DRAM") as dram, \
             tc.tile_pool(name="msg", bufs=5) as mp, \
             tc.tile_pool(name="midx", bufs=2) as ip, \
             tc.tile_pool(name="marr", bufs=2) as ap_, \
             tc.tile_pool(name="stl", bufs=12) as sp, \
             tc.tile_pool(name="xblk", bufs=2) as xp, \
             tc.tile_pool(name="ep", bufs=3) as epp, \
             tc.tile_pool(name="agg", bufs=3, space="PSUM") as pp, \
             tc.tile_pool(name="eps", bufs=2, space="PSUM") as pp2:

            iota_t = cp.tile([128, 128], dt)
            nc.sync.dma_start(iota_t[:], iota_in[:])
            ident_t = cp.tile([128, 128], dt)
            nc.sync.dma_start(ident_t[:], ident_in[:])
            W0_t = cp.tile([IN_FEATS, F], dt)
            nc.sync.dma_start(W0_t[:], W0_in[:])
            W1_t = cp.tile([F, F], dt)
            nc.sync.dma_start(W1_t[:], W1_in[:])
            W2_t = cp.tile([F, F], dt)
            nc.sync.dma_start(W2_t[:], W2_in[:])
            b0_t = cp.tile([F, 1], dt)
            nc.sync.dma_start(b0_t[:], b0_in[:])
            b1_t = cp.tile([F, 1], dt)
            nc.sync.dma_start(b1_t[:], b1_in[:])
            b2_t = cp.tile([F, 1], dt)
            nc.sync.dma_start(b2_t[:], b2_in[:])

            tbl = []
            for l in range(3):
                tb = dram.tile([NROWS, F], dt, tag=f"t{l}")
                tbl.append(tb)

            # ---- Layer-1 projection: t0 = X @ W0 ----
            for b in range(NBLK):
                xb = xp.tile([IN_FEATS, 128], dt, tag="xb")
                nc.sync.dma_start(xb[:], xT_in[:, b * 128:(b + 1) * 128])
                yp = pp2.tile([128, F], dt, tag="pj")
                nc.tensor.matmul(yp[:], xb[:], W0_t[:], start=True, stop=True)
                ys = epp.tile([128, F], dt, tag="ysb")
                nc.vector.tensor_copy(ys[:], yp[:])
                nc.sync.dma_start(tbl[0][b * 128:(b + 1) * 128, :], ys[:])

            # ---- Layers ----
            tile_ctr = 0
            for l in range(3):
                table = tbl[l]
                views = {"A": table[0:HSPLIT, :], "B": table[HSPLIT:NROWS, :]}
                msgs = {}
                arrs = {}
                for s in ("A", "B"):
                    T = sched[s][2]
                    n_chunks = (T + TPC - 1) // TPC
                    msgs[s] = []
                    arrs[s] = []
                    for ch in range(n_chunks):
                        nt = min(TPC, T - ch * TPC)
                        idx_t = ip.tile([128, nt * 8], mybir.dt.int16, tag="idx" + s)
                        nc.sync.dma_start(idx_t[:], meta_in["idx" + s][:, ch * TPC * 8: ch * TPC * 8 + nt * 8])
                        m3 = ap_.tile([128, 3 * nt], dt, tag="m3" + s)
                        nc.sync.dma_start(m3[:], meta_in["meta" + s][:, 3 * ch * TPC: 3 * ch * TPC + 3 * nt])
                        sl = m3[:, 0:nt]
                        iv = m3[:, nt:2 * nt]
                        nv = m3[:, 2 * nt:3 * nt]
                        msg = mp.tile([128, nt, F], dt, tag="msg" + s)
                        if _os.environ.get("SKIP_GATHER") != "1":
                            nc.gpsimd.dma_gather(
                                msg[:], views[s], idx_t[:],
                                num_idxs=nt * 128, num_idxs_reg=nt * 128,
                                elem_size=F, single_packet=False)
                        else:
                            nc.vector.memset(msg[:, 0, :], 0.0)
                        msgs[s].append(msg)
                        arrs[s].append((sl, iv, nv))

                for b in range(NBLK):
                    refs = []
                    for s in ("A", "B"):
                        tiles, tstarts, _T = sched[s]
                        for t in range(int(tstarts[b]), int(tstarts[b] + tiles[b])):
                            refs.append((s, t // TPC, t % TPC))
                    agg = pp.tile([128, F], dt, tag="agg")
                    nt_b = len(refs)
                    if _os.environ.get("SKIP_AGG") == "1":
                        refs = refs[:1]
                    for i, (s, ch, col) in enumerate(refs):
                        sl, iv, nv = arrs[s][ch]
                        S = sp.tile([128, 128], dt, tag="S")
                        tile_ctr += 1
                        if tile_ctr % ACT_EVERY == 0:
                            S1 = sp.tile([128, 128], dt, tag="S1")
                            nc.scalar.activation(
                                S1[:], iota_t[:], mybir.ActivationFunctionType.Abs,
                                bias=sl[:, col:col + 1], scale=-1.0)
                            nc.scalar.activation(
                                S[:], S1[:], mybir.ActivationFunctionType.Relu,
                                bias=iv[:, col:col + 1], scale=nv[:, col:col + 1])
                        else:
                            nc.vector.tensor_scalar(
                                S[:], iota_t[:], sl[:, col:col + 1], iv[:, col:col + 1],
                                mybir.AluOpType.is_equal, mybir.AluOpType.mult)
                        nc.tensor.matmul(agg[:], S[:], msgs[s][ch][:, col, :],
                                         start=(i == 0), stop=(i == nt_b - 1))

                    # epilogue
                    t0 = epp.tile([128, F], dt, tag="t0")
                    if nt_b == 0:
                        nc.vector.memset(t0[:], 0.0)
                    else:
                        nc.vector.tensor_copy(t0[:], agg[:])
                    t0T = pp2.tile([F, 128], dt, tag="t0T")
                    nc.tensor.transpose(t0T[:], t0[:], ident_t[:])
                    rows = slice(b * 128, (b + 1) * 128)
                    if l == 0:
                        hT = epp.tile([F, 128], dt, tag="hT")
                        nc.scalar.activation(hT[:], t0T[:], mybir.ActivationFunctionType.Relu,
                                             bias=b0_t[:, 0:1], scale=1.0)
                        yT = pp2.tile([F, 128], dt, tag="pj")
                        nc.tensor.matmul(yT[:], W1_t[:], hT[:], start=True, stop=True)
                        yTs = epp.tile([F, 128], dt, tag="yTs")
                        nc.vector.tensor_copy(yTs[:], yT[:])
                        yps = pp2.tile([128, F], dt, tag="pj")
                        nc.tensor.transpose(yps[:], yTs[:], ident_t[0:F, 0:F])
                        ysb = epp.tile([128, F], dt, tag="ysb")
                        nc.vector.tensor_copy(ysb[:], yps[:])
                        nc.sync.dma_start(tbl[1][rows, :], ysb[:])
                    elif l == 1:
                        hT = epp.tile([F, 128], dt, tag="hT")
                        nc.scalar.activation(hT[:], t0T[:], mybir.ActivationFunctionType.Relu,
                                             bias=b1_t[:, 0:1], scale=1.0)
                        hps = pp2.tile([128, F], dt, tag="pj")
                        nc.tensor.transpose(hps[:], hT[:], ident_t[0:F, 0:F])
                        hsb = epp.tile([128, F], dt, tag="ysb")
                        nc.vector.tensor_copy(hsb[:], hps[:])
                        nc.sync.dma_start(tbl[2][rows, :], hsb[:])
                    else:
                        # out = aggT.T @ W2p + b2: project the (normalized) agg
                        aT = epp.tile([F, 128], dt, tag="hT")
                        nc.vector.tensor_copy(aT[:], t0T[:])
                        oT = pp2.tile([F, 128], dt, tag="pj")
                        nc.tensor.matmul(oT[:], W2_t[:], aT[:], start=True, stop=True)
                        oTb = epp.tile([F, 128], dt, tag="yTs")
                        nc.scalar.activation(oTb[:], oT[:], mybir.ActivationFunctionType.Identity,
                                             bias=b2_t[:, 0:1], scale=1.0)
                        ops_ = pp2.tile([128, F], dt, tag="pj")
                        nc.tensor.transpose(ops_[:], oTb[:], ident_t[0:F, 0:F])
                        osb = epp.tile([128, F], dt, tag="ysb")
                        nc.vector.tensor_copy(osb[:], ops_[:])
                        nc.sync.dma_start(out[rows, :], osb[:])

    nc.compile()
    return nc


def kernel(features, src, dst, W0, b0, W1, b1, W2, b2):
    features = np.asarray(features, dtype=np.float32)
    src = np.asarray(src).astype(np.int64)
    dst = np.asarray(dst).astype(np.int64)
    in_map, sched = _prep(features, src, dst,
                          np.asarray(W0), np.asarray(b0), np.asarray(W1),
                          np.asarray(b1), np.asarray(W2), np.asarray(b2))
    key = (sched["A"][2], sched["B"][2],
           tuple(sched["A"][0].tolist()), tuple(sched["B"][0].tolist()))
    if _cache.get("key") != key:
        _cache["nc"] = _build(sched)
        _cache["key"] = key
    nc = _cache["nc"]
    res = run_bass_kernel_spmd(nc, [in_map], core_ids=[0])
    full = res.results[0]["out"]
    return np.ascontiguousarray(full[:N_NODES, :N_CLASSES])
